# revision 49
# baseline (speedup 1.0000x reference)
"""Trainium2 Bass kernel for nn_ContrastiveLoss3DTo2D.

Reference computation (B=256, D=1024, margin=0.2):
    scores[i, j] = dot(im[j], s[i, j])                    # [B, B]
    cost_s  = sum_i relu(margin + max_{j!=i} scores[i,j] - scores[i,i])
    cost_im = sum_j relu(margin + max_{i!=j} scores[i,j] - scores[j,j])
    loss = cost_s + cost_im

Sharding: s (and the score matrix) is sharded along i across 8 cores
(32 rows each); im is replicated. Inputs are cast to fp16 on the host
(loss tolerance is 2e-2; fp16 keeps the loss error ~1e-5 rel) which
halves HBM traffic — the binding constraint — and doubles DVE
elementwise-mul throughput. Each core streams its 16 MB shard on the
Sync HWDGE ring (~52 us at ~330 GB/s) while DVE and ACT split the
64 dot-product reductions per core (see the per-row mode comments in
_build_nc; accumulate paths run 1 elem/cycle on every engine, so the
split across two engines is what hides compute under the stream).

Column layout: j = 2p + u (partition p in [0,128), u in {0,1}) so each
DMA descriptor is a contiguous 4 KB run (two adjacent j rows of D).
Per-core reductions produce colmax[256] (diag masked), diag[32], and
rowcost[32], packed via 32x32 stream transposes into a single [4,128]
fp32 tensor written with one 4-descriptor DMA (per-partition-column
outputs would emit hundreds of 4-byte descriptors that crawl for >10us).
The host combines per-core partials exactly as relu/max commute.

Measured on trn2 (NTFF profile, core 0): 77.4 us vs the 136.5 us
staged baseline; ~6.5 us NEFF preamble + ~5.6 us first-data latency +
~61 us compute span (DVE/ACT both ~95% busy; stream itself ends at
~52 us) + ~4.4 us epilogue, output DMA and HBM write receipt.
"""

import numpy as np

B = 256
D = 1024
M = 8            # cores
BL = B // M      # 32 local rows per core
P = 128          # SBUF partitions
U = 2            # j = 2p + u column interleave
MARGIN = 0.2
NEG = -1.0e30    # diagonal mask value
NEG_INIT = -3.0e38

_NC = None


def _build_nc():
    import concourse.bacc as bacc
    from concourse import mybir
    from concourse.tile import TileContext

    f32 = mybir.dt.float32
    f16 = mybir.dt.float16
    add = mybir.AluOpType.add
    mult = mybir.AluOpType.mult
    amax = mybir.AluOpType.max

    nc = bacc.Bacc(None, target_bir_lowering=False, debug=False)
    # s is host-reshaped to [row, p, (u d)]: partition line = rows
    # 2p,2p+1 of im-space = one contiguous 4KB DMA descriptor; row
    # chunks are plain first-dim slices. The ramp's half-row loads
    # slice the free dim, which is equally legal.
    im_d = nc.declare_dram_parameter("im", [B, D], f16, isOutput=False)
    s_d = nc.declare_dram_parameter("s", [BL, P, U * D], f16, isOutput=False)
    mt_d = nc.declare_dram_parameter("mask_t_neg", [P, U * BL], f32, isOutput=False)
    nr_d = nc.declare_dram_parameter("neg_rows", [BL, B], f32, isOutput=False)
    er_d = nc.declare_dram_parameter("eye_rows", [BL, B], f32, isOutput=False)
    o_d = nc.declare_dram_parameter("out", [4, P], f32, isOutput=True)

    with TileContext(nc) as tc:
        with (
            tc.tile_pool(name="const", bufs=1) as cpool,
            tc.tile_pool(name="sload", bufs=5) as spool,
            tc.tile_pool(name="scratch", bufs=2) as prpool,
            tc.tile_pool(name="prods", bufs=12) as mpool,
            tc.tile_pool(name="small", bufs=1) as smpool,
            tc.psum_pool(name="pgarbF", bufs=1) as gpoolF,
            tc.psum_pool(name="pgarbA", bufs=1) as gpoolA,
        ):
            # im packed as [p, u*D + d] with j = 2p + u, matching s tiles.
            # First on the Sync ring so compute can start ~11us in; the
            # epilogue-only masks ride the scalar ring (ACT is idle early).
            im_t = cpool.tile([P, U * D], f16, tag="im")
            im_r = im_d[:].rearrange("(p u) d -> p (u d)", p=P)
            nc.sync.dma_start(out=im_t[:], in_=im_r[:])
            # Row 0 opens as two 256KB half-loads: the u0 half on the
            # scalar ring (its DGE, the ACT engine, is idle this early —
            # mid-stream chunks must NOT ride it), the u1 half on Sync
            # right behind im. The first mul then waits on half the
            # bytes, pulling both DVE's and ACT's start earlier.
            row0 = s_d[0:1].rearrange("r p f -> p (r f)")
            s_t0a = spool.tile([P, D], f16, tag="s0")
            nc.scalar.dma_start(out=s_t0a[:], in_=row0[:, 0:D])
            s_t0b = spool.tile([P, D], f16, tag="s0")
            nc.sync.dma_start(out=s_t0b[:], in_=row0[:, D:U * D])
            mt_t = cpool.tile([P, U * BL], f32, tag="maskT")
            nc.scalar.dma_start(out=mt_t[:], in_=mt_d[:])
            nr_t = cpool.tile([BL, B], f32, tag="negrows")
            nc.scalar.dma_start(out=nr_t[:], in_=nr_d[:])
            er_t = cpool.tile([BL, B], f32, tag="eyerows")
            nc.scalar.dma_start(out=er_t[:], in_=er_d[:])

            # scores^T: partition p, free column u*BL + i  (j = 2p + u).
            # One accumulator per engine: a shared tile would serialize
            # DVE and ACT into convoys (every accum write is ordered
            # against the previous engine's write to the same tile).
            # Both start at 0; each column is written by exactly one
            # engine; the epilogue merges with a single add.
            scoresD = smpool.tile([P, U * BL], f32, tag="scoresD")
            scoresA = smpool.tile([P, U * BL], f32, tag="scoresA")
            nc.gpsimd.memset(scoresD[:], 0.0)
            nc.gpsimd.memset(scoresA[:], 0.0)
            scoresT = smpool.tile([P, U * BL], f32, tag="scoresT")

            # Ramped whole-row chunks for rows 1..31 (half-row units,
            # all even; row 0 was loaded above): tiny first so the ramp
            # trails DVE by <1.5us, tiny last so the stream tail is
            # short. Splitting rows into half-row DMAs (2KB descriptors)
            # doubled Sync desc-gen and lost ~3us net.
            chunk_halves = [2, 2, 2, 4, 4, 4, 8, 8, 8, 8, 6, 4, 2]
            assert sum(chunk_halves) == U * BL - U
            assert all(ch % 2 == 0 for ch in chunk_halves)

            # Per-ROW engine assignment, measured on HW:
            #   F-row: 2x DVE fused scalar_tensor_tensor (1223 + 140 ns
            #      each, 1 elem/cyc ALU-bound; garbage out -> PSUM).
            #   A-row: ONE DVE fp16 mul [P, 2*D] -> fp16 SBUF (2 elem/cyc,
            #      930 ns; covers both halves), then 2x ACT accum
            #      (1148 + 283 ns each, write-back -> PSUM bank).
            # 13 F-rows / 19 A-rows. Rows 0-2 are A so ACT's pipeline
            # starts as soon as the first mul lands instead of idling
            # behind an opening F-row. F is clustered at the END:
            # once the stream is done DVE self-accumulates the last rows
            # while ACT drains its backlog of earlier A-rows, instead of
            # DVE idling behind ACT. (GpSimd can't help: its
            # tensor_reduce is partition-axis only, and its big muls
            # poison DVE's 2-ports mode.)
            f_rows = {3, 5, 8, 11, 14, 17, 20, 23, 26, 28, 29, 30, 31}
            assert len(f_rows) == 13 and BL - 1 in f_rows

            # All s chunks ride the Sync HWDGE ring: the scalar ring's
            # DGE is the ACT engine, and ACT is ~95% busy accumulating —
            # chunks issued there arrive late and stall DVE.
            # row 0 compute (A-row by construction: 0 not in f_rows),
            # one [P, D] mul + accum per half so each starts as soon as
            # its half-load lands
            assert 0 not in f_rows
            for u, half in ((0, s_t0a), (1, s_t0b)):
                prod0 = mpool.tile([P, D], f16, tag="prodh")
                nc.vector.tensor_mul(
                    prod0[:], half[:], im_t[:, u * D:(u + 1) * D],
                )
                garbA = gpoolA.tile([P, D], f32, tag="garbA")
                nc.scalar.activation(
                    out=garbA[:],
                    in_=prod0[:],
                    func=mybir.ActivationFunctionType.Copy,
                    accum_out=scoresA[:, u * BL:u * BL + 1],
                )

            h0 = U
            for ci, nh in enumerate(chunk_halves):
                s_t = spool.tile([P, nh * D], f16, tag="s")
                if nh % 2 == 0:
                    r0, nr = h0 // U, nh // U
                    nc.sync.dma_start(
                        out=s_t[:, 0:nh * D].rearrange(
                            "p (r f) -> p r f", r=nr),
                        in_=s_d[r0:r0 + nr].rearrange("r p f -> p r f"),
                    )
                else:
                    nc.sync.dma_start(
                        out=s_t[:],
                        in_=s_d[h0 // U:h0 // U + 1].rearrange(
                            "r p f -> p (r f)")[:, (h0 % U) * D:(h0 % U + 1) * D],
                    )
                h = h0
                while h < h0 + nh:
                    i, u = h // U, h % U
                    off = (h - h0) * D
                    if i in f_rows:
                        garb = gpoolF.tile([P, D], f32, tag="garbF")
                        nc.vector.scalar_tensor_tensor(
                            out=garb[:],
                            in0=s_t[:, off:off + D],
                            scalar=1.0,
                            in1=im_t[:, u * D:(u + 1) * D],
                            op0=mult,
                            op1=mult,
                            accum_out=scoresD[:, u * BL + i:u * BL + i + 1],
                        )
                        h += 1
                    elif u == 0 and h + 1 < h0 + nh:
                        # whole A-row inside this chunk: one [P, 2D] mul
                        prod = mpool.tile([P, U * D], f16, tag="prod")
                        nc.vector.tensor_mul(
                            prod[:], s_t[:, off:off + U * D], im_t[:],
                        )
                        for uu in range(U):
                            garbA = gpoolA.tile([P, D], f32, tag="garbA")
                            nc.scalar.activation(
                                out=garbA[:],
                                in_=prod[:, uu * D:(uu + 1) * D],
                                func=mybir.ActivationFunctionType.Copy,
                                accum_out=scoresA[:, uu * BL + i:uu * BL + i + 1],
                            )
                        h += 2
                    else:
                        # split A-half (row 0's ramp halves)
                        prod = mpool.tile([P, D], f16, tag="prodh")
                        nc.vector.tensor_mul(
                            prod[:],
                            s_t[:, off:off + D],
                            im_t[:, u * D:(u + 1) * D],
                        )
                        garbA = gpoolA.tile([P, D], f32, tag="garbA")
                        nc.scalar.activation(
                            out=garbA[:],
                            in_=prod[:],
                            func=mybir.ActivationFunctionType.Copy,
                            accum_out=scoresA[:, u * BL + i:u * BL + i + 1],
                        )
                        h += 1
                h0 += nh

            # Merge the engine accumulators (disjoint columns, 0 elsewhere)
            nc.vector.tensor_add(scoresT[:], scoresD[:], scoresA[:])

            # Packed output tile: col 0/1 = colmax (u=0/1), col 2 = diag,
            # col 3 = rowcost. Transposed at the end into [4, 128]. The
            # memset covers the pad lanes the transposes read.
            out_t = smpool.tile([P, 32], f32, tag="out_t")
            nc.gpsimd.memset(out_t[:], 0.0)

            # Column maxima over local rows, diagonal masked to -1e30:
            # fused (scoresT + mask) then max-reduce.
            for u in range(U):
                cscr = prpool.tile([P, BL], f32, tag="cscr")
                nc.vector.tensor_add(
                    cscr[:],
                    scoresT[:, u * BL:(u + 1) * BL],
                    mt_t[:, u * BL:(u + 1) * BL],
                )
                nc.vector.reduce_max(
                    out_t[:, u:u + 1], cscr[:], axis=mybir.AxisListType.X
                )

            # Transpose scores^T -> rows [32, 256] via 32x32 stream blocks.
            # rows[i, u*128 + pp] = score(i, j=2*pp+u).
            rows = smpool.tile([BL, B], f32, tag="rows")
            for u in range(U):
                for k in range(P // 32):
                    nc.vector.transpose(
                        out=rows[0:BL, u * P + k * 32:u * P + (k + 1) * 32],
                        in_=scoresT[k * 32:(k + 1) * 32, u * BL:(u + 1) * BL],
                    )

            # rowmax (diag masked) and diag, both fused single passes.
            rowstat = smpool.tile([BL, 4], f32, tag="rowstat")
            rscr1 = prpool.tile([BL, B], f32, tag="rscr")
            nc.vector.tensor_add(rscr1[:], rows[:], nr_t[:])
            nc.vector.reduce_max(
                rowstat[:, 0:1], rscr1[:], axis=mybir.AxisListType.X
            )
            rscr2 = prpool.tile([BL, B], f32, tag="rscr")
            # diag = sum(rows * eye) fused in one pass
            nc.vector.scalar_tensor_tensor(
                out=rscr2[:],
                in0=rows[:],
                scalar=1.0,
                in1=er_t[:],
                op0=mult,
                op1=mult,
                accum_out=out_t[0:BL, 2:3],
            )
            # rowcost = relu(margin + rowmax - diag)
            nc.vector.tensor_sub(rowstat[:, 1:2], rowstat[:, 0:1], out_t[0:BL, 2:3])
            nc.vector.tensor_scalar(
                out=out_t[0:BL, 3:4], in0=rowstat[:, 1:2],
                scalar1=MARGIN, scalar2=0.0, op0=add, op1=amax,
            )

            # Pack: transpose out_t's first 4 columns into rows of outT,
            # then ONE 4-descriptor DMA (512B per partition line).
            outT = smpool.tile([32, P], f32, tag="outT")
            for k in range(P // 32):
                nc.vector.transpose(
                    out=outT[0:32, k * 32:(k + 1) * 32],
                    in_=out_t[k * 32:(k + 1) * 32, 0:32],
                )
            nc.scalar.dma_start(out=o_d[:], in_=outT[0:4, 0:P])

    nc.compile()
    return nc


def _get_nc():
    global _NC
    if _NC is None:
        _NC = _build_nc()
    return _NC


def _make_in_maps(im, s):
    im16 = im.astype(np.float16)
    s16 = s.astype(np.float16)
    il = np.arange(BL)
    # column q in `rows` layout: q = u*128 + pp  <->  j = 2*pp + u
    jq = 2 * (np.arange(B) % P) + (np.arange(B) // P)
    in_maps = []
    for c in range(M):
        jdiag = c * BL + il                      # global row index of local i
        mt = np.zeros((P, U * BL), np.float32)   # mt[p, u*BL+i]
        pd, ud = jdiag % P, jdiag // P
        # j = 2p+u == jdiag  =>  p = jdiag//2, u = jdiag%2
        mt[jdiag // 2, (jdiag % 2) * BL + il] = NEG
        nr = np.zeros((BL, B), np.float32)
        er = np.zeros((BL, B), np.float32)
        qdiag = (jdiag % 2) * P + jdiag // 2     # q with j(q) == jdiag
        nr[il, qdiag] = NEG
        er[il, qdiag] = 1.0
        in_maps.append({
            "im": im16,
            "s": s16[c * BL:(c + 1) * BL].reshape(BL, P, U * D),
            "mask_t_neg": mt,
            "neg_rows": nr,
            "eye_rows": er,
        })
    return in_maps


def _combine(results):
    colmax = np.full(B, -np.inf, np.float32)
    rowcosts = np.empty(B, np.float32)
    diag = np.empty(B, np.float32)
    for c in range(M):
        o = results[c]["out"]                    # [4, 128] fp32
        cm = np.stack([o[0], o[1]], axis=1).ravel()   # j = 2p+u
        colmax = np.maximum(colmax, cm)
        diag[c * BL:(c + 1) * BL] = o[2, :BL]
        rowcosts[c * BL:(c + 1) * BL] = o[3, :BL]
    cost_im = np.maximum(np.float32(MARGIN) + colmax - diag, np.float32(0.0))
    loss = rowcosts.sum(dtype=np.float32) + cost_im.sum(dtype=np.float32)
    return np.array(loss, dtype=np.float32)


def _run(im, s, **spmd_kwargs):
    from concourse.bass_utils import run_bass_kernel_spmd

    im = np.ascontiguousarray(np.asarray(im), dtype=np.float32)
    s = np.ascontiguousarray(np.asarray(s), dtype=np.float32)
    nc = _get_nc()
    res = run_bass_kernel_spmd(nc, _make_in_maps(im, s), list(range(M)),
                               **spmd_kwargs)
    return _combine(res.results), res


def kernel(im, s):
    loss, _ = _run(im, s)
    return loss


# revision 50
# speedup vs baseline: 1.1884x; 1.1884x over previous
"""Trainium2 Bass kernel for nn_ContrastiveLoss3DTo2D.

Reference computation (B=256, D=1024, margin=0.2):
    scores[i, j] = dot(im[j], s[i, j])                    # [B, B]
    cost_s  = sum_i relu(margin + max_{j!=i} scores[i,j] - scores[i,i])
    cost_im = sum_j relu(margin + max_{i!=j} scores[i,j] - scores[j,j])
    loss = cost_s + cost_im

Sharding: s (and the score matrix) is sharded along i across 8 cores
(32 rows each); im is replicated. Inputs are cast to fp16 on the host
(loss tolerance is 2e-2; fp16 keeps the loss error ~1e-5 rel) which
halves HBM traffic — the binding constraint — and doubles DVE
elementwise-mul throughput. Each core streams its 16 MB shard on the
Sync HWDGE ring (~52 us at ~330 GB/s) while DVE and ACT split the
64 dot-product reductions per core (see the per-row mode comments in
_build_nc; accumulate paths run 1 elem/cycle on every engine, so the
split across two engines is what hides compute under the stream).

Column layout: j = 2p + u (partition p in [0,128), u in {0,1}) so each
DMA descriptor is a contiguous 4 KB run (two adjacent j rows of D).
Per-core reductions produce colmax[256] (diag masked), diag[32], and
rowcost[32], packed via 32x32 stream transposes into a single [4,128]
fp32 tensor written with one 4-descriptor DMA (per-partition-column
outputs would emit hundreds of 4-byte descriptors that crawl for >10us).
The host combines per-core partials exactly as relu/max commute.

Measured on trn2 (NTFF profile, core 0): 77.5 us vs the 136.5 us
staged baseline; ~6.5 us NEFF preamble + ~6.9 us first-data latency +
~60 us compute span (DVE/ACT both ~95% busy; stream itself ends at
~52 us) + ~4.4 us epilogue, output DMA and HBM write receipt.
"""

import numpy as np

B = 256
D = 1024
M = 8            # cores
BL = B // M      # 32 local rows per core
P = 128          # SBUF partitions
U = 2            # j = 2p + u column interleave
MARGIN = 0.2
NEG = -1.0e30    # diagonal mask value
NEG_INIT = -3.0e38

_NC = None


def _build_nc():
    import concourse.bacc as bacc
    from concourse import mybir
    from concourse.tile import TileContext

    f32 = mybir.dt.float32
    f16 = mybir.dt.float16
    add = mybir.AluOpType.add
    mult = mybir.AluOpType.mult
    amax = mybir.AluOpType.max

    nc = bacc.Bacc(None, target_bir_lowering=False, debug=False)
    # s is host-reshaped to [row, p, (u d)]: partition line = rows
    # 2p,2p+1 of im-space = one contiguous 4KB DMA descriptor; row
    # chunks are plain first-dim slices. The ramp's half-row loads
    # slice the free dim, which is equally legal.
    im_d = nc.declare_dram_parameter("im", [B, D], f16, isOutput=False)
    s_d = nc.declare_dram_parameter("s", [BL, P, U * D], f16, isOutput=False)
    mt_d = nc.declare_dram_parameter("mask_t_neg", [P, U * BL], f32, isOutput=False)
    nr_d = nc.declare_dram_parameter("neg_rows", [BL, B], f32, isOutput=False)
    er_d = nc.declare_dram_parameter("eye_rows", [BL, B], f32, isOutput=False)
    o_d = nc.declare_dram_parameter("out", [4, P], f32, isOutput=True)

    with TileContext(nc) as tc:
        with (
            tc.tile_pool(name="const", bufs=1) as cpool,
            tc.tile_pool(name="sload", bufs=5) as spool,
            tc.tile_pool(name="scratch", bufs=2) as prpool,
            tc.tile_pool(name="prods", bufs=12) as mpool,
            tc.tile_pool(name="small", bufs=1) as smpool,
            tc.psum_pool(name="pgarbF", bufs=1) as gpoolF,
            tc.psum_pool(name="pgarbA", bufs=1) as gpoolA,
        ):
            # im packed as [p, u*D + d] with j = 2p + u, matching s tiles.
            # First on the Sync ring so compute can start ~11us in; the
            # epilogue-only masks ride the scalar ring (ACT is idle early).
            im_t = cpool.tile([P, U * D], f16, tag="im")
            im_r = im_d[:].rearrange("(p u) d -> p (u d)", p=P)
            nc.sync.dma_start(out=im_t[:], in_=im_r[:])
            # Row 0 opens on the scalar ring, in parallel with im on the
            # Sync ring: ACT's DGE is idle this early, so row 0 lands at
            # ~9us and both DVE and ACT start ~2.5us sooner. (Mid-stream
            # chunks must NOT ride this ring — ACT is busy by then.)
            s_t0 = spool.tile([P, U * D], f16, tag="s")
            nc.scalar.dma_start(
                out=s_t0[:, 0:U * D].rearrange("p (r f) -> p r f", r=1),
                in_=s_d[0:1].rearrange("r p f -> p r f"),
            )
            mt_t = cpool.tile([P, U * BL], f32, tag="maskT")
            nc.scalar.dma_start(out=mt_t[:], in_=mt_d[:])
            nr_t = cpool.tile([BL, B], f32, tag="negrows")
            nc.scalar.dma_start(out=nr_t[:], in_=nr_d[:])
            er_t = cpool.tile([BL, B], f32, tag="eyerows")
            nc.scalar.dma_start(out=er_t[:], in_=er_d[:])

            # scores^T: partition p, free column u*BL + i  (j = 2p + u).
            # One accumulator per engine: a shared tile would serialize
            # DVE and ACT into convoys (every accum write is ordered
            # against the previous engine's write to the same tile).
            # Both start at 0; each column is written by exactly one
            # engine; the epilogue merges with a single add.
            scoresD = smpool.tile([P, U * BL], f32, tag="scoresD")
            scoresA = smpool.tile([P, U * BL], f32, tag="scoresA")
            nc.gpsimd.memset(scoresD[:], 0.0)
            nc.gpsimd.memset(scoresA[:], 0.0)
            scoresT = smpool.tile([P, U * BL], f32, tag="scoresT")

            # Ramped whole-row chunks (half-row units, all even): tiny
            # first so the first mul starts ~13us in, tiny last so the
            # stream tail is short. Splitting the ramp finer (2KB
            # descriptors) doubled Sync desc-gen and lost ~3us net.
            chunk_halves = [2, 2, 2, 4, 8, 8, 8, 8, 8, 6, 4, 2]
            assert sum(chunk_halves) == U * BL - U
            assert all(ch % 2 == 0 for ch in chunk_halves)

            # Per-ROW engine assignment, measured on HW:
            #   F-row: 2x DVE fused scalar_tensor_tensor (1223 + 140 ns
            #      each, 1 elem/cyc ALU-bound; garbage out -> PSUM).
            #   A-row: ONE DVE fp16 mul [P, 2*D] -> fp16 SBUF (2 elem/cyc,
            #      930 ns; covers both halves), then 2x ACT accum
            #      (1148 + 283 ns each, write-back -> PSUM bank).
            # 13 F-rows / 19 A-rows. Rows 0-2 are A so ACT's pipeline
            # starts as soon as the first mul lands instead of idling
            # behind an opening F-row. F is clustered at the END:
            # once the stream is done DVE self-accumulates the last rows
            # while ACT drains its backlog of earlier A-rows, instead of
            # DVE idling behind ACT. (GpSimd can't help: its
            # tensor_reduce is partition-axis only, and its big muls
            # poison DVE's 2-ports mode.)
            f_rows = {3, 5, 8, 11, 14, 17, 20, 23, 26, 28, 29, 30, 31}
            assert len(f_rows) == 13 and BL - 1 in f_rows

            # All s chunks ride the Sync HWDGE ring: the scalar ring's
            # DGE is the ACT engine, and ACT is ~95% busy accumulating —
            # chunks issued there arrive late and stall DVE.
            # row 0 compute (A-row by construction: 0 not in f_rows)
            assert 0 not in f_rows
            prod0 = mpool.tile([P, U * D], f16, tag="prod")
            nc.vector.tensor_mul(prod0[:], s_t0[:], im_t[:])
            for u in range(U):
                garbA = gpoolA.tile([P, D], f32, tag="garbA")
                nc.scalar.activation(
                    out=garbA[:],
                    in_=prod0[:, u * D:(u + 1) * D],
                    func=mybir.ActivationFunctionType.Copy,
                    accum_out=scoresA[:, u * BL:u * BL + 1],
                )

            h0 = U
            for ci, nh in enumerate(chunk_halves):
                s_t = spool.tile([P, nh * D], f16, tag="s")
                if nh % 2 == 0:
                    r0, nr = h0 // U, nh // U
                    nc.sync.dma_start(
                        out=s_t[:, 0:nh * D].rearrange(
                            "p (r f) -> p r f", r=nr),
                        in_=s_d[r0:r0 + nr].rearrange("r p f -> p r f"),
                    )
                else:
                    nc.sync.dma_start(
                        out=s_t[:],
                        in_=s_d[h0 // U:h0 // U + 1].rearrange(
                            "r p f -> p (r f)")[:, (h0 % U) * D:(h0 % U + 1) * D],
                    )
                h = h0
                while h < h0 + nh:
                    i, u = h // U, h % U
                    off = (h - h0) * D
                    if i in f_rows:
                        garb = gpoolF.tile([P, D], f32, tag="garbF")
                        nc.vector.scalar_tensor_tensor(
                            out=garb[:],
                            in0=s_t[:, off:off + D],
                            scalar=1.0,
                            in1=im_t[:, u * D:(u + 1) * D],
                            op0=mult,
                            op1=mult,
                            accum_out=scoresD[:, u * BL + i:u * BL + i + 1],
                        )
                        h += 1
                    elif u == 0 and h + 1 < h0 + nh:
                        # whole A-row inside this chunk: one [P, 2D] mul
                        prod = mpool.tile([P, U * D], f16, tag="prod")
                        nc.vector.tensor_mul(
                            prod[:], s_t[:, off:off + U * D], im_t[:],
                        )
                        for uu in range(U):
                            garbA = gpoolA.tile([P, D], f32, tag="garbA")
                            nc.scalar.activation(
                                out=garbA[:],
                                in_=prod[:, uu * D:(uu + 1) * D],
                                func=mybir.ActivationFunctionType.Copy,
                                accum_out=scoresA[:, uu * BL + i:uu * BL + i + 1],
                            )
                        h += 2
                    else:
                        # split A-half (row 0's ramp halves)
                        prod = mpool.tile([P, D], f16, tag="prodh")
                        nc.vector.tensor_mul(
                            prod[:],
                            s_t[:, off:off + D],
                            im_t[:, u * D:(u + 1) * D],
                        )
                        garbA = gpoolA.tile([P, D], f32, tag="garbA")
                        nc.scalar.activation(
                            out=garbA[:],
                            in_=prod[:],
                            func=mybir.ActivationFunctionType.Copy,
                            accum_out=scoresA[:, u * BL + i:u * BL + i + 1],
                        )
                        h += 1
                h0 += nh

            # Merge the engine accumulators (disjoint columns, 0 elsewhere)
            nc.vector.tensor_add(scoresT[:], scoresD[:], scoresA[:])

            # Packed output tile: col 0/1 = colmax (u=0/1), col 2 = diag,
            # col 3 = rowcost. Transposed at the end into [4, 128]. The
            # memset covers the pad lanes the transposes read.
            out_t = smpool.tile([P, 32], f32, tag="out_t")
            nc.gpsimd.memset(out_t[:], 0.0)

            # Column maxima over local rows, diagonal masked to -1e30:
            # fused (scoresT + mask) then max-reduce.
            for u in range(U):
                cscr = prpool.tile([P, BL], f32, tag="cscr")
                nc.vector.tensor_add(
                    cscr[:],
                    scoresT[:, u * BL:(u + 1) * BL],
                    mt_t[:, u * BL:(u + 1) * BL],
                )
                nc.vector.reduce_max(
                    out_t[:, u:u + 1], cscr[:], axis=mybir.AxisListType.X
                )

            # Transpose scores^T -> rows [32, 256] via 32x32 stream blocks.
            # rows[i, u*128 + pp] = score(i, j=2*pp+u).
            rows = smpool.tile([BL, B], f32, tag="rows")
            for u in range(U):
                for k in range(P // 32):
                    nc.vector.transpose(
                        out=rows[0:BL, u * P + k * 32:u * P + (k + 1) * 32],
                        in_=scoresT[k * 32:(k + 1) * 32, u * BL:(u + 1) * BL],
                    )

            # rowmax (diag masked) and diag, both fused single passes.
            rowstat = smpool.tile([BL, 4], f32, tag="rowstat")
            rscr1 = prpool.tile([BL, B], f32, tag="rscr")
            nc.vector.tensor_add(rscr1[:], rows[:], nr_t[:])
            nc.vector.reduce_max(
                rowstat[:, 0:1], rscr1[:], axis=mybir.AxisListType.X
            )
            rscr2 = prpool.tile([BL, B], f32, tag="rscr")
            # diag = sum(rows * eye) fused in one pass
            nc.vector.scalar_tensor_tensor(
                out=rscr2[:],
                in0=rows[:],
                scalar=1.0,
                in1=er_t[:],
                op0=mult,
                op1=mult,
                accum_out=out_t[0:BL, 2:3],
            )
            # rowcost = relu(margin + rowmax - diag)
            nc.vector.tensor_sub(rowstat[:, 1:2], rowstat[:, 0:1], out_t[0:BL, 2:3])
            nc.vector.tensor_scalar(
                out=out_t[0:BL, 3:4], in0=rowstat[:, 1:2],
                scalar1=MARGIN, scalar2=0.0, op0=add, op1=amax,
            )

            # Pack: transpose out_t's first 4 columns into rows of outT,
            # then ONE 4-descriptor DMA (512B per partition line).
            outT = smpool.tile([32, P], f32, tag="outT")
            for k in range(P // 32):
                nc.vector.transpose(
                    out=outT[0:32, k * 32:(k + 1) * 32],
                    in_=out_t[k * 32:(k + 1) * 32, 0:32],
                )
            nc.scalar.dma_start(out=o_d[:], in_=outT[0:4, 0:P])

    nc.compile()
    return nc


def _get_nc():
    global _NC
    if _NC is None:
        _NC = _build_nc()
    return _NC


def _make_in_maps(im, s):
    im16 = im.astype(np.float16)
    s16 = s.astype(np.float16)
    il = np.arange(BL)
    # column q in `rows` layout: q = u*128 + pp  <->  j = 2*pp + u
    jq = 2 * (np.arange(B) % P) + (np.arange(B) // P)
    in_maps = []
    for c in range(M):
        jdiag = c * BL + il                      # global row index of local i
        mt = np.zeros((P, U * BL), np.float32)   # mt[p, u*BL+i]
        pd, ud = jdiag % P, jdiag // P
        # j = 2p+u == jdiag  =>  p = jdiag//2, u = jdiag%2
        mt[jdiag // 2, (jdiag % 2) * BL + il] = NEG
        nr = np.zeros((BL, B), np.float32)
        er = np.zeros((BL, B), np.float32)
        qdiag = (jdiag % 2) * P + jdiag // 2     # q with j(q) == jdiag
        nr[il, qdiag] = NEG
        er[il, qdiag] = 1.0
        in_maps.append({
            "im": im16,
            "s": s16[c * BL:(c + 1) * BL].reshape(BL, P, U * D),
            "mask_t_neg": mt,
            "neg_rows": nr,
            "eye_rows": er,
        })
    return in_maps


def _combine(results):
    colmax = np.full(B, -np.inf, np.float32)
    rowcosts = np.empty(B, np.float32)
    diag = np.empty(B, np.float32)
    for c in range(M):
        o = results[c]["out"]                    # [4, 128] fp32
        cm = np.stack([o[0], o[1]], axis=1).ravel()   # j = 2p+u
        colmax = np.maximum(colmax, cm)
        diag[c * BL:(c + 1) * BL] = o[2, :BL]
        rowcosts[c * BL:(c + 1) * BL] = o[3, :BL]
    cost_im = np.maximum(np.float32(MARGIN) + colmax - diag, np.float32(0.0))
    loss = rowcosts.sum(dtype=np.float32) + cost_im.sum(dtype=np.float32)
    return np.array(loss, dtype=np.float32)


def _run(im, s, **spmd_kwargs):
    from concourse.bass_utils import run_bass_kernel_spmd

    im = np.ascontiguousarray(np.asarray(im), dtype=np.float32)
    s = np.ascontiguousarray(np.asarray(s), dtype=np.float32)
    nc = _get_nc()
    res = run_bass_kernel_spmd(nc, _make_in_maps(im, s), list(range(M)),
                               **spmd_kwargs)
    return _combine(res.results), res


def kernel(im, s):
    loss, _ = _run(im, s)
    return loss
